# revision 17
# baseline (speedup 1.0000x reference)
"""GQA attention (B=2,S=2048,DIM=2048,H=32,KVH=8,HD=64) + RoPE, causal.

Distributed over 8 TRN2 NeuronCores: core = 4*batch + head_group.
Each core computes attention for its 8 q-heads (2 kv-heads) of one batch.
Q^T / K^T are produced directly by the projection matmuls (weights
stationary, x^T moving) so no transpose of Q/K is ever needed; RoPE is
applied in the transposed [hd, seq] layout with replicated cos/sin rows.
The causal mask is fused into the score matmul as an accumulated
(identity x lower-triangular -240) product.  The output projection is
computed per chunk as partial products against the core's own 512 rows
of wo, then summed + distributed with a per-chunk ReduceScatter.
Host-side work is layout-only: weight column/row permutations, batch
split, cos/sin row replication, and concatenation of per-core outputs.
"""
import numpy as np

import concourse.bass as bass
import concourse.bacc as bacc
import concourse.tile as tile
from concourse.tile import add_dep_helper
import concourse.mybir as mybir
from concourse import bass_utils


def _ensure_axon_hooks_shim():
    """bass_utils imports antenv.axon_hooks when BASS_TRACE is set; the
    module is absent in some images. Provide a no-op shim so tracing env
    vars cannot crash the run."""
    import sys, types
    try:
        import antenv  # noqa
        if "antenv.axon_hooks" in sys.modules:
            return
        import importlib
        try:
            importlib.import_module("antenv.axon_hooks")
            return
        except ImportError:
            pass
        mod = types.ModuleType("antenv.axon_hooks")
        mod._hook = None
        mod.get_axon_ntff_profile_hook = lambda: mod._hook

        def set_axon_ntff_profile_hook(h):
            mod._hook = h
        mod.set_axon_ntff_profile_hook = set_axon_ntff_profile_hook
        sys.modules["antenv.axon_hooks"] = mod
        antenv.axon_hooks = mod
    except Exception:
        pass


_ensure_axon_hooks_shim()

F32 = mybir.dt.float32
BF16 = mybir.dt.bfloat16

B, S, DIM = 2, 2048, 2048
H, KVH, HD = 32, 8, 64
N_CORES = 8
GROUPS = [[0, 1, 2, 3], [4, 5, 6, 7]]
NCH = 4            # sequence chunks (queries) of 512
CHUNK = S // NCH   # 512
SEQT = S // 128    # 16 seq tiles
DT = DIM // 128    # 16 contraction tiles
# q-head slot order inside a core: slot s holds local q-head s//2 + 4*(s%2),
# so slot parity == local kv-head index (kv = local_head // 4).
SLOT_TO_LOCAL = [s // 2 + 4 * (s % 2) for s in range(8)]
# rope pair permutation within one head: 16-interleaved halves so the
# (a, b) cross-swap is a within-32-quadrant partition shuffle:
# [a0..a15, b0..b15, a16..a31, b16..b31] where a_i = dim 2i, b_i = dim 2i+1
HD_PERM = np.concatenate([np.arange(0, 32, 2), np.arange(1, 32, 2),
                          np.arange(32, 64, 2), np.arange(33, 64, 2)])
SWAP_MASK = list(range(16, 32)) + list(range(0, 16))
MASK_NEG = -240.0


def _build():
    nc = bacc.Bacc("TRN2", target_bir_lowering=False, debug=False,
                   num_devices=N_CORES)
    x_d = nc.dram_tensor("x", [S, DIM], F32, kind="ExternalInput")
    wq_d = nc.dram_tensor("wq", [DIM, 512], F32, kind="ExternalInput")
    wkv_d = nc.dram_tensor("wkv", [DIM, 256], F32, kind="ExternalInput")
    wo_d = nc.dram_tensor("wo", [512, DIM], F32, kind="ExternalInput")
    cosr_d = nc.dram_tensor("cosr", [128, S], F32, kind="ExternalInput")
    sinr_d = nc.dram_tensor("sinr", [128, S], F32, kind="ExternalInput")
    out_d = nc.dram_tensor("out", [CHUNK, DIM], F32, kind="ExternalOutput")

    Exp = mybir.ActivationFunctionType.Exp

    with tile.TileContext(nc) as tc:
        with tc.tile_pool(name="dram", bufs=1, space="DRAM") as dram, \
             tc.tile_pool(name="wpool", bufs=1) as wpool:
            # ---- DRAM scratch ----
            xbf = dram.tile([S, DIM], BF16)
            partial = dram.tile([NCH, CHUNK, DIM], BF16)
            rsout = dram.tile([NCH, 128, DIM], BF16)

            # ---- persistent SBUF ----
            wq_sb = wpool.tile([128, DT, 512], BF16)
            wkv_sb = wpool.tile([128, DT, 256], BF16)
            wo_sb = wpool.tile([128, 4, DIM], BF16)
            cosr_sb = wpool.tile([128, S], BF16)
            sinr_sb = wpool.tile([128, S], BF16)
            kt_sb = wpool.tile([128, S], BF16)        # K^T (kv0|kv1) full seq
            v_sb = wpool.tile([128, SEQT, 130], BF16)  # [V0|1|V1|1] per key tile
            iden_sb = wpool.tile([128, 128], BF16)     # identity
            ltneg_sb = wpool.tile([128, 128], BF16)    # MASK_NEG strictly lower

            # constants: ones columns of V_aug; identity; lower-tri mask
            nc.gpsimd.memset(v_sb[:, :, 64:65], 1.0)
            nc.gpsimd.memset(v_sb[:, :, 129:130], 1.0)
            nc.gpsimd.memset(iden_sb[:], 1.0)
            nc.gpsimd.affine_select(
                out=iden_sb[:], in_=iden_sb[:],
                compare_op=mybir.AluOpType.is_equal,
                fill=0.0, base=0,
                pattern=[[-1, 128]], channel_multiplier=1,
            )
            nc.gpsimd.memset(ltneg_sb[:], MASK_NEG)
            nc.gpsimd.affine_select(
                out=ltneg_sb[:], in_=ltneg_sb[:],
                compare_op=mybir.AluOpType.is_ge,
                fill=0.0, base=-1,
                pattern=[[-1, 128]], channel_multiplier=1,
            )

            # preload the exp table set so it doesn't stall the first QK
            warm = wpool.tile([128, 1], F32)
            nc.gpsimd.memset(warm[:], 0.0)
            nc.scalar.activation(warm[:], warm[:], Exp)

            with tc.tile_pool(name="xio", bufs=2) as xio, \
                 tc.tile_pool(name="asb", bufs=2) as asb, \
                 tc.tile_pool(name="bgps", bufs=2, space="PSUM") as bgps, \
                 tc.tile_pool(name="apsum", bufs=1, space="PSUM") as apsum:

                def load_weights():
                    """weights + rope tables on the scalar queue (idle until
                    the first exp) so they never contend with x staging."""
                    for kt in range(DT):
                        wt = xio.tile([128, 512 + 256], F32, tag="wf", bufs=2,
                                      name="wt")
                        nc.scalar.dma_start(wt[:, 0:512],
                                            wq_d[kt * 128:(kt + 1) * 128, :])
                        nc.scalar.dma_start(wt[:, 512:768],
                                            wkv_d[kt * 128:(kt + 1) * 128, :])
                        nc.vector.tensor_copy(wq_sb[:, kt, :], wt[:, 0:512])
                        nc.vector.tensor_copy(wkv_sb[:, kt, :], wt[:, 512:768])
                    for r in range(2):
                        rt = xio.tile([128, DIM], F32, tag="xf", bufs=2,
                                      name="rt")
                        nc.scalar.dma_start(
                            rt[:], (cosr_d if r == 0 else sinr_d)[:])
                        nc.vector.tensor_copy(
                            (cosr_sb if r == 0 else sinr_sb)[:], rt[:])
                    for sp in range(4):
                        wof = xio.tile([128, DIM], F32, tag="xf", bufs=2,
                                       name="wof")
                        nc.scalar.dma_start(wof[:],
                                            wo_d[sp * 128:(sp + 1) * 128, :])
                        nc.vector.tensor_copy(wo_sb[:, sp, :], wof[:])

                def stage_x(c):
                    """x chunk c: f32 -> bf16 -> DRAM (for xbar transpose)."""
                    for tt in range(4):
                        gt = 4 * c + tt
                        xf = xio.tile([128, DIM], F32, tag="xf", bufs=2,
                                      name="xf")
                        nc.gpsimd.dma_start(xf[:], x_d[gt * 128:(gt + 1) * 128, :])
                        xb = xio.tile([128, DIM], BF16, tag="xb", bufs=2,
                                      name="xb")
                        eng = nc.vector if tt % 2 == 0 else nc.gpsimd
                        eng.tensor_copy(xb[:], xf[:])
                        nc.gpsimd.dma_start(xbf[gt * 128:(gt + 1) * 128, :], xb[:])

                def transpose_x(c):
                    xT = xio.tile([128, DT, CHUNK], BF16, tag="xT", bufs=2,
                                  name="xT")
                    for dt in range(DT):
                        nc.sync.dma_start_transpose(
                            xT[:, dt, :],
                            xbf[c * CHUNK:(c + 1) * CHUNK, dt * 128:(dt + 1) * 128])
                    return xT

                def rope(c, ps, out):
                    """ps: [128, 512] f32 PSUM (per 32-quadrant: rows 0:16 = a,
                    16:32 = b); out: [128, 512] bf16 SBUF slice."""
                    cw = slice(c * CHUNK, (c + 1) * CHUNK)
                    t1 = asb.tile([128, CHUNK], BF16, tag="t1", bufs=2,
                                  name="t1")
                    psw = asb.tile([128, CHUNK], F32, tag="psw", bufs=1,
                                   name="psw")
                    t2 = asb.tile([128, CHUNK], BF16, tag="t2", bufs=2,
                                  name="t2")
                    nc.vector.tensor_mul(t1[:], ps[:], cosr_sb[:, cw])
                    nc.vector.stream_shuffle(psw[:], ps[:], SWAP_MASK)
                    nc.vector.tensor_mul(t2[:], psw[:], sinr_sb[:, cw])
                    nc.vector.tensor_add(out, t1[:], t2[:])

                def proj(c, xT):
                    qt = xio.tile([128, 4, CHUNK], BF16, tag="qt", bufs=2,
                                  name="qt")
                    for sp in range(4):
                        qps = bgps.tile([128, CHUNK], F32, tag="bg", bufs=2,
                                        name="qps")
                        for dt in range(DT):
                            nc.tensor.matmul(
                                qps[:], wq_sb[:, dt, sp * 128:(sp + 1) * 128],
                                xT[:, dt, :], start=(dt == 0), stop=(dt == DT - 1))
                        rope(c, qps, qt[:, sp, :])
                    kps = bgps.tile([128, CHUNK], F32, tag="bg", bufs=2,
                                    name="kps")
                    for dt in range(DT):
                        nc.tensor.matmul(
                            kps[:], wkv_sb[:, dt, 0:128],
                            xT[:, dt, :], start=(dt == 0), stop=(dt == DT - 1))
                    rope(c, kps, kt_sb[:, c * CHUNK:(c + 1) * CHUNK])
                    vps = bgps.tile([128, CHUNK], F32, tag="bg", bufs=2,
                                    name="vps")
                    for tt in range(4):
                        gt = 4 * c + tt
                        for dt in range(DT):
                            nc.tensor.matmul(
                                vps[:, tt * 128:(tt + 1) * 128],
                                xT[:, dt, tt * 128:(tt + 1) * 128],
                                wkv_sb[:, dt, 128:256],
                                start=(dt == 0), stop=(dt == DT - 1))
                        nc.vector.tensor_copy(v_sb[:, gt, 0:64],
                                              vps[:, tt * 128:tt * 128 + 64])
                        nc.vector.tensor_copy(v_sb[:, gt, 65:129],
                                              vps[:, tt * 128 + 64:tt * 128 + 128])
                    return qt

                def emit_scale_wo_rs(pc, pstages, precipb):
                    """normalize stages, wo partial matmuls, ReduceScatter."""
                    recflat = asb.tile([1, 8 * CHUNK], BF16, tag="recflat",
                                       bufs=1, name="recflat")
                    nc.gpsimd.dma_start(
                        recflat[0:1, :].rearrange("p (s q) -> p s q", s=8),
                        precipb[:, None, :])
                    rec_bc = asb.tile([64, 4, 1024], BF16, tag="recbc",
                                      bufs=1, name="rec_bc")
                    sts = asb.tile([128, 4, CHUNK], BF16, tag="sts", bufs=2,
                                   name="sts")
                    for sp in range(4):
                        for j in range(2):
                            s = 2 * sp + j
                            nc.gpsimd.partition_broadcast(
                                rec_bc[0:64, sp, 512 * j:512 * (j + 1)],
                                recflat[0:1, 512 * s:512 * (s + 1)])
                        stp = asb.tile([64, 1024], BF16, tag="stp", bufs=1,
                                       name="stp")
                        nc.vector.tensor_mul(
                            stp[:], pstages[sp][0:64, :], rec_bc[0:64, sp, :])
                        for j in range(2):
                            nc.gpsimd.dma_start(
                                sts[64 * j:64 * (j + 1), sp, :],
                                stp[0:64, 512 * j:512 * (j + 1)])
                    pdmas = []
                    for qs in range(4):
                        ostage = asb.tile([128, DIM], BF16, tag="ost", bufs=2,
                                          name="ostage")
                        for nb in range(4):
                            wop = bgps.tile([128, 512], F32, tag="bg", bufs=2,
                                            name="wop")
                            for sp in range(4):
                                nc.tensor.matmul(
                                    wop[:],
                                    sts[:, sp, qs * 128:(qs + 1) * 128],
                                    wo_sb[:, sp, nb * 512:(nb + 1) * 512],
                                    start=(sp == 0), stop=(sp == 3))
                            nc.vector.tensor_copy(
                                ostage[:, nb * 512:(nb + 1) * 512], wop[:])
                        pdmas.append(nc.gpsimd.dma_start(
                            partial[pc, qs * 128:(qs + 1) * 128, :], ostage[:]))
                    cc = nc.gpsimd.collective_compute(
                        "ReduceScatter", mybir.AluOpType.add,
                        replica_groups=GROUPS,
                        ins=[partial[pc][:, :].opt()],
                        outs=[rsout[pc][:, :].opt()])
                    for d in pdmas:
                        add_dep_helper(cc.ins, d.ins, sync=True,
                                       reason="RS waits partial DMAs")
                    cc_insts.append(cc)

                # ---- fused main loop ----
                cc_insts = []
                stage_x(0)
                xT = transpose_x(0)
                load_weights()
                pending = None
                for c in range(NCH):
                    qt = proj(c, xT)
                    if c + 1 < NCH:
                        stage_x(c + 1)
                        xT = transpose_x(c + 1)
                    denoms8 = asb.tile([8, CHUNK], BF16, tag="denoms", bufs=2,
                                       name="denoms8")
                    stgs = []
                    for sp in range(4):
                        aps = apsum.tile([128, 1024], F32, tag="aps", bufs=1,
                                         name="aps")
                        for kt in range(4 * c + 4):
                            vs = max(0, 128 * kt - CHUNK * c)
                            diag = kt >= 4 * c
                            spt = apsum.tile([128, 1024], F32, tag="sps",
                                             bufs=2, name="spt")
                            for j in range(2):
                                nc.tensor.matmul(
                                    spt[:, 512 * j + vs:512 * j + 512],
                                    kt_sb[64 * j:64 * j + 64, kt * 128:(kt + 1) * 128],
                                    qt[64 * j:64 * j + 64, sp, vs:CHUNK],
                                    start=True, stop=not diag)
                            if diag:
                                for j in range(2):
                                    nc.tensor.matmul(
                                        spt[:, 512 * j + vs:512 * j + vs + 128],
                                        iden_sb[:], ltneg_sb[:],
                                        start=False, stop=True,
                                        skip_group_check=True)
                            pt = asb.tile([128, 1024], BF16, tag="pT", bufs=5,
                                          name="pt")
                            nc.scalar.activation(
                                pt[:].rearrange("p (h q) -> p h q", h=2)[:, :, vs:512],
                                spt[:].rearrange("p (h q) -> p h q", h=2)[:, :, vs:512],
                                Exp, scale=0.125)
                            for j in range(2):
                                nc.tensor.matmul(
                                    aps[0:65, 512 * j + vs:512 * j + 512],
                                    v_sb[:, kt, 65 * j:65 * j + 65],
                                    pt[:, 512 * j + vs:512 * j + 512],
                                    start=(kt == 0), stop=(kt == 4 * c + 3))
                        stg = asb.tile([128, 1024], BF16, tag="stage", bufs=5,
                                       name="stg")
                        nc.vector.tensor_copy(stg[0:65, :], aps[0:65, :])
                        for j in range(2):
                            s = 2 * sp + j
                            nc.gpsimd.dma_start(
                                denoms8[s:s + 1, :],
                                stg[64:65, 512 * j:512 * (j + 1)])
                        stgs.append(stg)
                        if sp == 0 and pending is not None:
                            emit_scale_wo_rs(*pending)
                            pending = None
                    recip8 = asb.tile([8, CHUNK], F32, tag="recip", bufs=2,
                                      name="recip8")
                    nc.vector.reciprocal(recip8[:], denoms8[:])
                    recip8b = asb.tile([8, CHUNK], BF16, tag="recipb", bufs=2,
                                       name="recip8b")
                    nc.gpsimd.tensor_copy(recip8b[:], recip8[:])
                    pending = (c, stgs, recip8b)
                emit_scale_wo_rs(*pending)

                # ---- drain RS outputs -> f32 out rows (after all CCs) ----
                for pc in range(NCH):
                    rsb = asb.tile([128, DIM], BF16, tag="ost", bufs=2,
                                   name="rsb")
                    rd = nc.gpsimd.dma_start(rsb[:], rsout[pc][:, :])
                    add_dep_helper(rd.ins, cc_insts[pc].ins, sync=True,
                                   reason="rsout read waits its RS")
                    rf = xio.tile([128, DIM], F32, tag="xf", bufs=2, name="rf")
                    nc.vector.tensor_copy(rf[:], rsb[:])
                    nc.gpsimd.dma_start(out_d[pc * 128:(pc + 1) * 128, :], rf[:])

    nc.finalize()
    return nc


_NC_CACHE = None


def _get_nc():
    global _NC_CACHE
    if _NC_CACHE is None:
        _NC_CACHE = _build()
    return _NC_CACHE


def _shard_inputs(x, wq, wk, wv, wo, freqs_cos, freqs_sin):
    """Pure layout work: slice batch, pick each core's heads, permute rope
    pairs within each head, shard wo rows per core, replicate cos/sin."""
    x = np.ascontiguousarray(np.asarray(x, dtype=np.float32))
    wq = np.asarray(wq, dtype=np.float32)
    wk = np.asarray(wk, dtype=np.float32)
    wv = np.asarray(wv, dtype=np.float32)
    wo = np.asarray(wo, dtype=np.float32)
    cos = np.asarray(freqs_cos, dtype=np.float32)
    sin = np.asarray(freqs_sin, dtype=np.float32)

    # replicated rope tables matching the transposed Q^T/K^T row layout:
    # row r (within a 64-row slot block, w = r % 64, quadrant q2 = w // 16):
    # freq index i = (q2 // 2) * 16 + (w % 16); a-halves (q2 even) get -sin.
    cosr = np.empty((128, S), dtype=np.float32)
    sinr = np.empty((128, S), dtype=np.float32)
    for r in range(128):
        w = r % 64
        q2 = w // 16
        i = (q2 // 2) * 16 + (w % 16)
        cosr[r] = cos[:, i]
        sinr[r] = (-1.0 if q2 % 2 == 0 else 1.0) * sin[:, i]
    cosr = np.ascontiguousarray(cosr)
    sinr = np.ascontiguousarray(sinr)

    in_maps = []
    for core in range(N_CORES):
        b, g = core // 4, core % 4
        wq_cols = []
        wo_rows = []
        for s_ in range(8):
            h = 8 * g + SLOT_TO_LOCAL[s_]
            wq_cols.append(wq[:, 64 * h + HD_PERM])
            wo_rows.append(wo[64 * h:64 * (h + 1), :])
        wq_s = np.ascontiguousarray(np.concatenate(wq_cols, axis=1))
        wo_s = np.ascontiguousarray(np.concatenate(wo_rows, axis=0))
        wk_cols = [wk[:, 64 * (2 * g + j) + HD_PERM] for j in range(2)]
        wv_cols = wv[:, 64 * 2 * g: 64 * (2 * g + 2)]
        wkv_s = np.ascontiguousarray(
            np.concatenate(wk_cols + [wv_cols], axis=1))
        in_maps.append({
            "x": x[b], "wq": wq_s, "wkv": wkv_s, "wo": wo_s,
            "cosr": cosr, "sinr": sinr,
        })
    return in_maps


def kernel(x, wq, wk, wv, wo, freqs_cos, freqs_sin, mask=None, start_pos=0,
           **_unused):
    nc = _get_nc()
    in_maps = _shard_inputs(x, wq, wk, wv, wo, freqs_cos, freqs_sin)
    res = bass_utils.run_bass_kernel_spmd(
        nc, in_maps, core_ids=list(range(N_CORES)))
    out = np.empty((B, S, DIM), dtype=np.float32)
    for core in range(N_CORES):
        b, g = core // 4, core % 4
        co = res.results[core]["out"]
        for c in range(NCH):
            out[b, CHUNK * c + 128 * g: CHUNK * c + 128 * (g + 1), :] = \
                co[128 * c:128 * (c + 1), :]
    return out


# revision 20
# speedup vs baseline: 1.2116x; 1.2116x over previous
"""GQA attention (B=2,S=2048,DIM=2048,H=32,KVH=8,HD=64) + RoPE, causal.

Distributed over 8 TRN2 NeuronCores: core = 4*batch + head_group.
Each core computes attention for its 8 q-heads (2 kv-heads) of one batch.
Q^T / K^T are produced directly by the projection matmuls (weights
stationary, x^T moving) so no transpose of Q/K is ever needed; RoPE is
applied in the transposed [hd, seq] layout with replicated cos/sin rows.
The causal mask is fused into the score matmul as an accumulated
(identity x lower-triangular -240) product.  The output projection is
computed per chunk as partial products against the core's own 512 rows
of wo, then summed + distributed with a per-chunk ReduceScatter.
Host-side work is layout-only: weight column/row permutations, batch
split, cos/sin row replication, and concatenation of per-core outputs.
"""
import numpy as np

import concourse.bass as bass
import concourse.bacc as bacc
import concourse.tile as tile
from concourse.tile import add_dep_helper
import concourse.mybir as mybir
from concourse import bass_utils


def _ensure_axon_hooks_shim():
    """bass_utils imports antenv.axon_hooks when BASS_TRACE is set; the
    module is absent in some images. Provide a no-op shim so tracing env
    vars cannot crash the run."""
    import sys, types
    try:
        import antenv  # noqa
        if "antenv.axon_hooks" in sys.modules:
            return
        import importlib
        try:
            importlib.import_module("antenv.axon_hooks")
            return
        except ImportError:
            pass
        mod = types.ModuleType("antenv.axon_hooks")
        mod._hook = None
        mod.get_axon_ntff_profile_hook = lambda: mod._hook

        def set_axon_ntff_profile_hook(h):
            mod._hook = h
        mod.set_axon_ntff_profile_hook = set_axon_ntff_profile_hook
        sys.modules["antenv.axon_hooks"] = mod
        antenv.axon_hooks = mod
    except Exception:
        pass


_ensure_axon_hooks_shim()

F32 = mybir.dt.float32
BF16 = mybir.dt.bfloat16

B, S, DIM = 2, 2048, 2048
H, KVH, HD = 32, 8, 64
N_CORES = 8
GROUPS = [[0, 1, 2, 3], [4, 5, 6, 7]]
NCH = 4            # sequence chunks (queries) of 512
CHUNK = S // NCH   # 512
SEQT = S // 128    # 16 seq tiles
DT = DIM // 128    # 16 contraction tiles
# q-head slot order inside a core: slot s holds local q-head s//2 + 4*(s%2),
# so slot parity == local kv-head index (kv = local_head // 4).
SLOT_TO_LOCAL = [s // 2 + 4 * (s % 2) for s in range(8)]
# rope pair permutation within one head: 16-interleaved halves so the
# (a, b) cross-swap is a within-32-quadrant partition shuffle:
# [a0..a15, b0..b15, a16..a31, b16..b31] where a_i = dim 2i, b_i = dim 2i+1
HD_PERM = np.concatenate([np.arange(0, 32, 2), np.arange(1, 32, 2),
                          np.arange(32, 64, 2), np.arange(33, 64, 2)])
SWAP_MASK = list(range(16, 32)) + list(range(0, 16))
MASK_NEG = -240.0


def _build():
    nc = bacc.Bacc("TRN2", target_bir_lowering=False, debug=False,
                   num_devices=N_CORES)
    x_d = nc.dram_tensor("x", [S, DIM], F32, kind="ExternalInput")
    wq_d = nc.dram_tensor("wq", [DIM, 512], F32, kind="ExternalInput")
    wkv_d = nc.dram_tensor("wkv", [DIM, 256], F32, kind="ExternalInput")
    wo_d = nc.dram_tensor("wo", [512, DIM], F32, kind="ExternalInput")
    cosr_d = nc.dram_tensor("cosr", [128, S], F32, kind="ExternalInput")
    sinr_d = nc.dram_tensor("sinr", [128, S], F32, kind="ExternalInput")
    out_d = nc.dram_tensor("out", [CHUNK, DIM], F32, kind="ExternalOutput")

    Exp = mybir.ActivationFunctionType.Exp

    with tile.TileContext(nc) as tc:
        with tc.tile_pool(name="dram", bufs=1, space="DRAM") as dram, \
             tc.tile_pool(name="wpool", bufs=1) as wpool:
            # ---- DRAM scratch ----
            xbf = dram.tile([S, DIM], BF16)
            partial = dram.tile([NCH, CHUNK, DIM], BF16)
            rsout = dram.tile([NCH, 128, DIM], BF16)

            # ---- persistent SBUF ----
            wq_sb = wpool.tile([128, DT, 512], BF16)
            wkv_sb = wpool.tile([128, DT, 256], BF16)
            wo_sb = wpool.tile([128, 4, DIM], BF16)
            cosr_sb = wpool.tile([128, S], BF16)
            sinr_sb = wpool.tile([128, S], BF16)
            kt_sb = wpool.tile([128, S], BF16)        # K^T (kv0|kv1) full seq
            v_sb = wpool.tile([128, SEQT, 130], BF16)  # [V0|1|V1|1] per key tile
            iden_sb = wpool.tile([128, 128], BF16)     # identity
            ltneg_sb = wpool.tile([128, 128], BF16)    # MASK_NEG strictly lower

            # constants: ones columns of V_aug; identity; lower-tri mask
            nc.gpsimd.memset(v_sb[:, :, 64:65], 1.0)
            nc.gpsimd.memset(v_sb[:, :, 129:130], 1.0)
            nc.gpsimd.memset(iden_sb[:], 1.0)
            nc.gpsimd.affine_select(
                out=iden_sb[:], in_=iden_sb[:],
                compare_op=mybir.AluOpType.is_equal,
                fill=0.0, base=0,
                pattern=[[-1, 128]], channel_multiplier=1,
            )
            nc.gpsimd.memset(ltneg_sb[:], MASK_NEG)
            nc.gpsimd.affine_select(
                out=ltneg_sb[:], in_=ltneg_sb[:],
                compare_op=mybir.AluOpType.is_ge,
                fill=0.0, base=-1,
                pattern=[[-1, 128]], channel_multiplier=1,
            )

            # preload the exp table set so it doesn't stall the first QK
            warm = wpool.tile([128, 1], F32)
            nc.gpsimd.memset(warm[:], 0.0)
            nc.scalar.activation(warm[:], warm[:], Exp)

            with tc.tile_pool(name="xio", bufs=2) as xio, \
                 tc.tile_pool(name="asb", bufs=2) as asb, \
                 tc.tile_pool(name="bgps", bufs=2, space="PSUM") as bgps, \
                 tc.tile_pool(name="apsum", bufs=1, space="PSUM") as apsum:

                def load_weights():
                    """weights + rope tables on the scalar queue (idle until
                    the first exp) so they never contend with x staging."""
                    for kt in range(DT):
                        wt = xio.tile([128, 512 + 256], F32, tag="wf", bufs=2,
                                      name="wt")
                        nc.scalar.dma_start(wt[:, 0:512],
                                            wq_d[kt * 128:(kt + 1) * 128, :])
                        nc.scalar.dma_start(wt[:, 512:768],
                                            wkv_d[kt * 128:(kt + 1) * 128, :])
                        nc.vector.tensor_copy(wq_sb[:, kt, :], wt[:, 0:512])
                        nc.vector.tensor_copy(wkv_sb[:, kt, :], wt[:, 512:768])
                    for r in range(2):
                        rt = xio.tile([128, DIM], F32, tag="xf", bufs=2,
                                      name="rt")
                        nc.scalar.dma_start(
                            rt[:], (cosr_d if r == 0 else sinr_d)[:])
                        nc.vector.tensor_copy(
                            (cosr_sb if r == 0 else sinr_sb)[:], rt[:])
                    for sp in range(4):
                        wof = xio.tile([128, DIM], F32, tag="xf", bufs=2,
                                       name="wof")
                        nc.scalar.dma_start(wof[:],
                                            wo_d[sp * 128:(sp + 1) * 128, :])
                        nc.vector.tensor_copy(wo_sb[:, sp, :], wof[:])

                def stage_x(c):
                    """x chunk c: f32 -> bf16 -> DRAM (for xbar transpose)."""
                    for tt in range(4):
                        gt = 4 * c + tt
                        xf = xio.tile([128, DIM], F32, tag="xf", bufs=2,
                                      name="xf")
                        nc.gpsimd.dma_start(xf[:], x_d[gt * 128:(gt + 1) * 128, :])
                        xb = xio.tile([128, DIM], BF16, tag="xb", bufs=2,
                                      name="xb")
                        nc.vector.tensor_copy(xb[:], xf[:])
                        nc.gpsimd.dma_start(xbf[gt * 128:(gt + 1) * 128, :], xb[:])

                def transpose_x(c):
                    xT = xio.tile([128, DT, CHUNK], BF16, tag="xT", bufs=2,
                                  name="xT")
                    for dt in range(DT):
                        nc.sync.dma_start_transpose(
                            xT[:, dt, :],
                            xbf[c * CHUNK:(c + 1) * CHUNK, dt * 128:(dt + 1) * 128])
                    return xT

                def rope(c, ps, out):
                    """ps: [128, 512] f32 PSUM (per 32-quadrant: rows 0:16 = a,
                    16:32 = b); out: [128, 512] bf16 SBUF slice."""
                    cw = slice(c * CHUNK, (c + 1) * CHUNK)
                    t1 = asb.tile([128, CHUNK], BF16, tag="t1", bufs=2,
                                  name="t1")
                    psw = asb.tile([128, CHUNK], F32, tag="psw", bufs=1,
                                   name="psw")
                    t2 = asb.tile([128, CHUNK], BF16, tag="t2", bufs=2,
                                  name="t2")
                    nc.vector.tensor_mul(t1[:], ps[:], cosr_sb[:, cw])
                    nc.vector.stream_shuffle(psw[:], ps[:], SWAP_MASK)
                    nc.vector.tensor_mul(t2[:], psw[:], sinr_sb[:, cw])
                    nc.vector.tensor_add(out, t1[:], t2[:])

                def proj(c, xT):
                    qt = xio.tile([128, 4, CHUNK], BF16, tag="qt", bufs=2,
                                  name="qt")
                    for sp in range(4):
                        qps = bgps.tile([128, CHUNK], F32, tag="bg", bufs=2,
                                        name="qps")
                        for dt in range(DT):
                            nc.tensor.matmul(
                                qps[:], wq_sb[:, dt, sp * 128:(sp + 1) * 128],
                                xT[:, dt, :], start=(dt == 0), stop=(dt == DT - 1))
                        rope(c, qps, qt[:, sp, :])
                    kps = bgps.tile([128, CHUNK], F32, tag="bg", bufs=2,
                                    name="kps")
                    for dt in range(DT):
                        nc.tensor.matmul(
                            kps[:], wkv_sb[:, dt, 0:128],
                            xT[:, dt, :], start=(dt == 0), stop=(dt == DT - 1))
                    rope(c, kps, kt_sb[:, c * CHUNK:(c + 1) * CHUNK])
                    vps = bgps.tile([128, CHUNK], F32, tag="bg", bufs=2,
                                    name="vps")
                    for tt in range(4):
                        gt = 4 * c + tt
                        for dt in range(DT):
                            nc.tensor.matmul(
                                vps[:, tt * 128:(tt + 1) * 128],
                                xT[:, dt, tt * 128:(tt + 1) * 128],
                                wkv_sb[:, dt, 128:256],
                                start=(dt == 0), stop=(dt == DT - 1))
                        nc.vector.tensor_copy(v_sb[:, gt, 0:64],
                                              vps[:, tt * 128:tt * 128 + 64])
                        nc.vector.tensor_copy(v_sb[:, gt, 65:129],
                                              vps[:, tt * 128 + 64:tt * 128 + 128])
                    return qt

                def emit_scale(pc, pstages, precipb):
                    """normalize + restack stages into the wo stationary."""
                    recflat = asb.tile([1, 8 * CHUNK], BF16, tag="recflat",
                                       bufs=1, name="recflat")
                    nc.scalar.dma_start(
                        recflat[0:1, :].rearrange("p (s q) -> p s q", s=8),
                        precipb[:, None, :])
                    rec_bc = asb.tile([64, 4, 1024], BF16, tag="recbc",
                                      bufs=1, name="rec_bc")
                    sts = asb.tile([128, 4, CHUNK], BF16, tag="sts", bufs=2,
                                   name="sts")
                    for sp in range(4):
                        for j in range(2):
                            s = 2 * sp + j
                            nc.gpsimd.partition_broadcast(
                                rec_bc[0:64, sp, 512 * j:512 * (j + 1)],
                                recflat[0:1, 512 * s:512 * (s + 1)])
                        stp = asb.tile([64, 1024], BF16, tag="stp", bufs=1,
                                       name="stp")
                        nc.vector.tensor_mul(
                            stp[:], pstages[sp][0:64, :], rec_bc[0:64, sp, :])
                        for j in range(2):
                            nc.gpsimd.dma_start(
                                sts[64 * j:64 * (j + 1), sp, :],
                                stp[0:64, 512 * j:512 * (j + 1)])
                    return sts

                def emit_wo(pc, sts, qs_list):
                    for qs in qs_list:
                        ostage = asb.tile([128, DIM], BF16, tag="ost", bufs=2,
                                          name="ostage")
                        for nb in range(4):
                            wop = bgps.tile([128, 512], F32, tag="bg", bufs=2,
                                            name="wop")
                            for sp in range(4):
                                nc.tensor.matmul(
                                    wop[:],
                                    sts[:, sp, qs * 128:(qs + 1) * 128],
                                    wo_sb[:, sp, nb * 512:(nb + 1) * 512],
                                    start=(sp == 0), stop=(sp == 3))
                            nc.vector.tensor_copy(
                                ostage[:, nb * 512:(nb + 1) * 512], wop[:])
                        pdmas[pc].append(nc.gpsimd.dma_start(
                            partial[pc, qs * 128:(qs + 1) * 128, :], ostage[:]))

                def emit_cc(pc):
                    cc = nc.gpsimd.collective_compute(
                        "ReduceScatter", mybir.AluOpType.add,
                        replica_groups=GROUPS,
                        ins=[partial[pc][:, :].opt()],
                        outs=[rsout[pc][:, :].opt()])
                    for d in pdmas[pc]:
                        add_dep_helper(cc.ins, d.ins, sync=True,
                                       reason="RS waits partial DMAs")
                    cc_insts.append(cc)

                # ---- fused main loop ----
                cc_insts = []
                pdmas = [[] for _ in range(NCH)]
                stage_x(0)
                xT = transpose_x(0)
                load_weights()
                pending = None
                psts = {}
                for c in range(NCH):
                    qt = proj(c, xT)
                    if c + 1 < NCH:
                        stage_x(c + 1)
                        xT = transpose_x(c + 1)
                    denoms8 = asb.tile([8, CHUNK], BF16, tag="denoms", bufs=2,
                                       name="denoms8")
                    stgs = []
                    for sp in range(4):
                        aps = apsum.tile([128, 1024], F32, tag="aps", bufs=1,
                                         name="aps")
                        for kt in range(4 * c + 4):
                            vs = max(0, 128 * kt - CHUNK * c)
                            diag = kt >= 4 * c
                            spt = apsum.tile([128, 1024], F32, tag="sps",
                                             bufs=2, name="spt")
                            for j in range(2):
                                nc.tensor.matmul(
                                    spt[:, 512 * j + vs:512 * j + 512],
                                    kt_sb[64 * j:64 * j + 64, kt * 128:(kt + 1) * 128],
                                    qt[64 * j:64 * j + 64, sp, vs:CHUNK],
                                    start=True, stop=not diag)
                            if diag:
                                for j in range(2):
                                    nc.tensor.matmul(
                                        spt[:, 512 * j + vs:512 * j + vs + 128],
                                        iden_sb[:], ltneg_sb[:],
                                        start=False, stop=True,
                                        skip_group_check=True)
                            pt = asb.tile([128, 1024], BF16, tag="pT", bufs=5,
                                          name="pt")
                            nc.scalar.activation(
                                pt[:].rearrange("p (h q) -> p h q", h=2)[:, :, vs:512],
                                spt[:].rearrange("p (h q) -> p h q", h=2)[:, :, vs:512],
                                Exp, scale=0.125)
                            for j in range(2):
                                nc.tensor.matmul(
                                    aps[0:65, 512 * j + vs:512 * j + 512],
                                    v_sb[:, kt, 65 * j:65 * j + 65],
                                    pt[:, 512 * j + vs:512 * j + 512],
                                    start=(kt == 0), stop=(kt == 4 * c + 3))
                        stg = asb.tile([128, 1024], BF16, tag="stage", bufs=5,
                                       name="stg")
                        nc.vector.tensor_copy(stg[0:65, :], aps[0:65, :])
                        for j in range(2):
                            s = 2 * sp + j
                            nc.scalar.dma_start(
                                denoms8[s:s + 1, :],
                                stg[64:65, 512 * j:512 * (j + 1)])
                        stgs.append(stg)
                        if pending is not None:
                            ppc = pending[0]
                            if sp == 0:
                                psts[ppc] = emit_scale(*pending)
                            elif sp == 1:
                                emit_wo(ppc, psts[ppc], [0, 1])
                            elif sp == 2:
                                emit_wo(ppc, psts[ppc], [2, 3])
                            else:
                                emit_cc(ppc)
                                pending = None
                    recip8 = asb.tile([8, CHUNK], F32, tag="recip", bufs=2,
                                      name="recip8")
                    nc.vector.reciprocal(recip8[:], denoms8[:])
                    recip8b = asb.tile([8, CHUNK], BF16, tag="recipb", bufs=2,
                                       name="recip8b")
                    nc.gpsimd.tensor_copy(recip8b[:], recip8[:])
                    pending = (c, stgs, recip8b)
                pc = pending[0]
                psts[pc] = emit_scale(*pending)
                emit_wo(pc, psts[pc], [0, 1, 2, 3])
                emit_cc(pc)

                # ---- drain RS outputs -> f32 out rows (after all CCs) ----
                for pc in range(NCH):
                    rsb = asb.tile([128, DIM], BF16, tag="ost", bufs=2,
                                   name="rsb")
                    rd = nc.gpsimd.dma_start(rsb[:], rsout[pc][:, :])
                    add_dep_helper(rd.ins, cc_insts[pc].ins, sync=True,
                                   reason="rsout read waits its RS")
                    rf = xio.tile([128, DIM], F32, tag="xf", bufs=2, name="rf")
                    nc.vector.tensor_copy(rf[:], rsb[:])
                    nc.gpsimd.dma_start(out_d[pc * 128:(pc + 1) * 128, :], rf[:])

    nc.finalize()
    return nc


_NC_CACHE = None


def _get_nc():
    global _NC_CACHE
    if _NC_CACHE is None:
        _NC_CACHE = _build()
    return _NC_CACHE


def _shard_inputs(x, wq, wk, wv, wo, freqs_cos, freqs_sin):
    """Pure layout work: slice batch, pick each core's heads, permute rope
    pairs within each head, shard wo rows per core, replicate cos/sin."""
    x = np.ascontiguousarray(np.asarray(x, dtype=np.float32))
    wq = np.asarray(wq, dtype=np.float32)
    wk = np.asarray(wk, dtype=np.float32)
    wv = np.asarray(wv, dtype=np.float32)
    wo = np.asarray(wo, dtype=np.float32)
    cos = np.asarray(freqs_cos, dtype=np.float32)
    sin = np.asarray(freqs_sin, dtype=np.float32)

    # replicated rope tables matching the transposed Q^T/K^T row layout:
    # row r (within a 64-row slot block, w = r % 64, quadrant q2 = w // 16):
    # freq index i = (q2 // 2) * 16 + (w % 16); a-halves (q2 even) get -sin.
    cosr = np.empty((128, S), dtype=np.float32)
    sinr = np.empty((128, S), dtype=np.float32)
    for r in range(128):
        w = r % 64
        q2 = w // 16
        i = (q2 // 2) * 16 + (w % 16)
        cosr[r] = cos[:, i]
        sinr[r] = (-1.0 if q2 % 2 == 0 else 1.0) * sin[:, i]
    cosr = np.ascontiguousarray(cosr)
    sinr = np.ascontiguousarray(sinr)

    in_maps = []
    for core in range(N_CORES):
        b, g = core // 4, core % 4
        wq_cols = []
        wo_rows = []
        for s_ in range(8):
            h = 8 * g + SLOT_TO_LOCAL[s_]
            wq_cols.append(wq[:, 64 * h + HD_PERM])
            wo_rows.append(wo[64 * h:64 * (h + 1), :])
        wq_s = np.ascontiguousarray(np.concatenate(wq_cols, axis=1))
        wo_s = np.ascontiguousarray(np.concatenate(wo_rows, axis=0))
        wk_cols = [wk[:, 64 * (2 * g + j) + HD_PERM] for j in range(2)]
        wv_cols = wv[:, 64 * 2 * g: 64 * (2 * g + 2)]
        wkv_s = np.ascontiguousarray(
            np.concatenate(wk_cols + [wv_cols], axis=1))
        in_maps.append({
            "x": x[b], "wq": wq_s, "wkv": wkv_s, "wo": wo_s,
            "cosr": cosr, "sinr": sinr,
        })
    return in_maps


def kernel(x, wq, wk, wv, wo, freqs_cos, freqs_sin, mask=None, start_pos=0,
           **_unused):
    nc = _get_nc()
    in_maps = _shard_inputs(x, wq, wk, wv, wo, freqs_cos, freqs_sin)
    res = bass_utils.run_bass_kernel_spmd(
        nc, in_maps, core_ids=list(range(N_CORES)))
    out = np.empty((B, S, DIM), dtype=np.float32)
    for core in range(N_CORES):
        b, g = core // 4, core % 4
        co = res.results[core]["out"]
        for c in range(NCH):
            out[b, CHUNK * c + 128 * g: CHUNK * c + 128 * (g + 1), :] = \
                co[128 * c:128 * (c + 1), :]
    return out
